# revision 1
# baseline (speedup 1.0000x reference)
"""Adaptive-threshold spike encoding on 8 TRN2 NeuronCores.

Math: the reference scans t=0..31 with
    acc += x; spike = acc >= thr_t; acc = spike ? 0 : acc; thr' = 0.9*thr + 0.1*|x|
With thr_t = x + 0.9^t*(0.5-x) (closed form), spike_t <=> acc_pre >= 0.9^t*(0.5-x)
where acc_pre = k*x (k = steps since last reset).  Dividing by x and scaling by
g^t (g = 1/0.9):  spike_t <=> m >= r  with  r = (0.5-x)/x  and  m = k*g^t,
updated as m' = select(m < r, g*m + q_t, 0),  q_t = g^(t+1).

TWO timesteps fuse into ONE custom-DVE op (one uop, 1x rate):
    M1 = select(m < r, g*m + q_t, 0);  m2 = select(M1 < r, g*M1 + q_t1, 0)
The pair state m2 three-way classifies both spike bits (for r > 0):
    m2 == 0        -> odd-step spike   (even 0, odd 1)
    m2 == q_t1     -> even-step spike  (M1 was reset; g*0+q_t1 is exact)
    m2 >= 2*q_t1   -> no spikes
(r <= 0, i.e. x >= 0.5, always has m2 == 0 and both bits set; host knows x.)
ScalarE compresses m2 to a uint8 code via Exp(-m2/q_t1 + 2.2):
m2=0 -> e^2.2=9.0 {9};  m2=q_t1 -> e^1.2=3.3 {3};  m2>=2q_t1 -> <=e^0.2 {0,1}
(large m2 decays to +0 — no overflow path).  Host decodes: odd = code>=6;
even = (2<code<6) | (odd & (x>=0.5)).

Sharding: feature dim across the 8 cores, 8192 features each, no comms.

Pipeline (v6): input DMA, recip/NR setup, pair 0, and pair 0's ACT+DMA all
run at eighth-tile (256-col) granularity so the DVE starts ~0.4us earlier
and the out-DMA stream ~1us earlier; ACT ops signal via attached then_inc
with one-deeper waits (no per-pair drain bubbles); last pair's ACT+DMA are
split 4-ways to drain the tail; mt/cd ring depth 8.
"""

import numpy as np
from contextlib import ExitStack
import concourse.bass as bass
import concourse.bacc as bacc
import concourse.mybir as mybir
from concourse import dve_ops as _dve_ops
from concourse.dve_spec import (
    C0, C1, C2, Spec, Src0, Src1, Zero, select, lower, minn, _has_src1,
)
from concourse.dve_uop import DveOpSpec
from concourse.bass_utils import run_bass_kernel_spmd

B = 32
F = 65536
T = 32
NCORES = 8
FS = F // NCORES  # 8192 features per core
FH = 4
FL = FS // FH  # 2048
P = B * FH  # 128 partitions
NPAIR = T // 2  # 16 step-pairs

G = 1.0 / 0.9
RING = 6

_cache: dict = {}


def _register(name, body, reference):
    for op in _dve_ops.OPS:
        if op.name == name:
            return op
    spec = Spec(body=body, reference=reference)
    shas = {}
    for ver in ("v3", "v4"):
        uops = lower(spec, ver=ver)
        shas[ver] = DveOpSpec(
            name=name, opcode=0, uops=uops, rd1_en=_has_src1(spec)
        ).sha(ver)
    op = _dve_ops.DveOp(name, spec, subdim=False, uops_sha=shas)
    _dve_ops.OPS.append(op)
    _dve_ops.CUSTOM_DVE_SPECS[name] = op.spec
    _dve_ops._SUB_OPCODE_FOR_NAME[name] = (
        _dve_ops._CUSTOM_DVE_ROW_BASE + len(_dve_ops.OPS) - 1
    )
    return op


def _nr_r_op():
    # r = min((0.5 - x) * y1*(2 - x*y1), 3e38) — fused Newton step + (0.5-x)
    # mult; the min maps a NaN from an x==0 seed to "never spikes" (DVE
    # min/max pick the non-NaN operand).
    # in0 = x, in1 = y1 (seed recip), s0 = 2.0, s1 = 0.5, imm2 = 3e38
    return _register(
        "RECIP_NR_R2_ANT",
        minn((C1 - Src0) * ((C0 - Src0 * Src1) * Src1), C2),
        lambda in0, in1, s0, s1, imm2: np.minimum(
            np.nan_to_num(
                (np.float32(s1) - in0.astype(np.float32))
                * ((np.float32(s0) - in0 * in1) * in1),
                nan=np.float32(imm2),
            ),
            np.float32(imm2),
        ).astype(np.float32),
    )


def _first_pair_op():
    # pair from a zero state, reading only r: M1 = select(0 < r, q_t, 0);
    # out = select(M1 < r, g*M1 + q_t1, 0).  in0 = r, s0 = g, s1 = q_t,
    # imm2 = q_t1.
    M1 = select(Zero < Src0, C1, Zero)
    return _register(
        "SPIKE_FIRST_PAIR_ANT",
        select(M1 < Src0, M1 * C0 + C2, Zero),
        lambda in0, in1, s0, s1, imm2: (
            lambda M1: np.where(
                M1 < in0, M1 * np.float32(s0) + np.float32(imm2), 0.0
            )
        )(np.where(0.0 < in0, np.float32(s1), 0.0).astype(np.float32)).astype(
            np.float32
        ),
    )


def _state2_op():
    # in0 = m, in1 = r, s0 = g, s1 = q_t, imm2 = q_{t+1}
    S1 = select(Src0 < Src1, Src0 * C0 + C1, Zero)
    return _register(
        "SPIKE_STATE2_ANT",
        select(S1 < Src1, S1 * C0 + C2, Zero),
        lambda in0, in1, s0, s1, imm2: (
            lambda M1: np.where(M1 < in1, M1 * np.float32(s0) + np.float32(imm2), 0.0)
        )(
            np.where(
                in0 < in1, in0.astype(np.float32) * np.float32(s0) + np.float32(s1), 0.0
            ).astype(np.float32)
        ).astype(np.float32),
    )


def _build(repeat: int = 1, ncol: int = 1, tails: int = 4,
           heads: int = 8, ring: int = 8) -> bass.Bass:
    """ncol: split every loop op into `ncol` column chunks (1 = baseline).
    tails/heads: extra split factor for the LAST/FIRST pair's ACT+DMA
    (tail drains the pipeline sooner; head starts the out-DMA stream
    sooner). ring: mt/cd tile ring depth (absorbs DMA jitter)."""
    f32 = mybir.dt.float32
    u8 = mybir.dt.uint8
    ALU = mybir.AluOpType
    op = _state2_op()
    nr_r = _nr_r_op()
    op0 = _first_pair_op()

    NP = NPAIR * repeat  # global step-pairs
    CW = FL // ncol  # column-chunk width

    nc = bacc.Bacc(target_bir_lowering=False)
    x = nc.declare_dram_parameter("x", [B, FS], f32, isOutput=False)
    out = nc.declare_dram_parameter("out", [B, NPAIR, FS], u8, isOutput=True)

    f32_tiles = ["x_sb", "inv_sb", "r_sb"] + [f"mt{i}" for i in range(ring)]
    u8_tiles = [f"cd{i}" for i in range(ring)]
    # ACT+DMA chunk counts per pair (head/tail pairs are split finer)
    counts = [
        ncol * (heads if p == 0 else tails if p == NP - 1 else 1)
        for p in range(NP)
    ]
    cum = [0]
    for c in counts:
        cum.append(cum[-1] + c)
    # DVE chunk counts per pair: global pair 0 is emitted as 8 eighth ops
    # interleaved with the per-eighth r setup (the out-DMA stream starts
    # ~2-3us earlier); all other pairs are ncol chunks
    dcnt = [8 if p == 0 else ncol for p in range(NP)]
    dcum = [0]
    for c in dcnt:
        dcum.append(dcum[-1] + c)
    sems = [f"sem_in{k}" for k in range(8)] + ["sem_m", "sem_cd", "sem_out"]
    with ExitStack() as ctx:
        tl = {n: ctx.enter_context(nc.sbuf_tensor(n, [P, FL], f32))
              for n in f32_tiles}
        tl.update({n: ctx.enter_context(nc.sbuf_tensor(n, [P, FL], u8))
                   for n in u8_tiles})
        bias_sb = ctx.enter_context(nc.sbuf_tensor("bias_sb", [P, 1], f32))
        sm = {n: ctx.enter_context(nc.semaphore(n)) for n in sems}
        x_sb, inv_sb, r_sb = tl["x_sb"], tl["inv_sb"], tl["r_sb"]
        sem_m, sem_cd, sem_out = sm["sem_m"], sm["sem_cd"], sm["sem_out"]
        block = ctx.enter_context(nc.Block())

        xv = x[:, :].rearrange("b (fh fl) -> (b fh) fl", fh=FH)
        ov = out[:, :, :].rearrange("b t (fh fl) -> t b fh fl", fh=FH)
        mts = [tl[f"mt{i}"] for i in range(ring)]
        cds = [tl[f"cd{i}"] for i in range(ring)]

        # pair p covers local steps 2p, 2p+1: q_even = g^(2p+1), q_odd = g^(2p+2)
        q_ev = [float(G ** ((2 * (p % NPAIR)) + 1)) for p in range(NP)]
        q_od = [float(G ** ((2 * (p % NPAIR)) + 2)) for p in range(NP)]
        last = NP - 1

        NQ = 8
        QW = FL // NQ
        sem_ins = [sm[f"sem_in{k}"] for k in range(NQ)]

        @block.sync
        def _(sync):
            for k in range(NQ):
                sync.dma_start(
                    out=x_sb[:, k * QW : (k + 1) * QW],
                    in_=xv[:, k * QW : (k + 1) * QW],
                ).then_inc(sem_ins[k], 16)
            for p in range(NP):
                nch = counts[p]
                w = FL // nch
                for h in range(nch):
                    hs = slice(h * w, (h + 1) * w)
                    # one-deeper: ACT chunk (p,h)'s completion is signalled
                    # by the following chunk's inc (trailing drain covers the
                    # final chunk)
                    sync.wait_ge(sem_cd, cum[p] + h + 2)
                    sync.dma_start(
                        out=ov[p % NPAIR][:, :, hs], in_=cds[p % ring][:, hs]
                    ).then_inc(sem_out, 16)

        @block.scalar
        def _(scalar):
            ACTF = mybir.ActivationFunctionType
            # dummy activation: pulls the exp table load off the critical
            # path (overlaps the input DMA / DVE setup)
            scalar.activation(
                cds[ring - 1][:, :1],
                mts[ring - 1][:, :1],
                ACTF.Exp,
                bias=bias_sb[:, :],
                scale=0.0,
            )
            for p in range(NP):
                nch = counts[p]
                w = FL // nch
                for h in range(nch):
                    hs = slice(h * w, (h + 1) * w)
                    # need the DVE chunk covering these columns, one-deeper
                    dve_h = h * dcnt[p] // nch
                    scalar.wait_ge(sem_m, dcum[p] + dve_h + 2)
                    if p >= ring:
                        # cds[p % ring][:, hs] free once pair p-ring's DMA
                        # chunks covering the same columns completed
                        pp = p - ring
                        idx_last = ((h + 1) * counts[pp] - 1) // nch
                        scalar.wait_ge(
                            sem_out, 16 * (cum[pp] + idx_last + 1)
                        )
                    scalar.activation(
                        cds[p % ring][:, hs],
                        mts[(p + 1) % ring][:, hs],
                        ACTF.Exp,
                        bias=bias_sb[:, :],
                        scale=-1.0 / q_od[p],
                    ).then_inc(sem_cd, 1)
            scalar.drain().then_inc(sem_cd, 1)

        @block.vector
        def _(vector):
            # setup: r = (0.5 - x) / x, m = 0 (consumed only by this engine)
            vector.memset(bias_sb[:, :], 2.2)
            for k in range(NQ):
                sl = slice(k * QW, (k + 1) * QW)
                vector.wait_ge(sem_ins[k], 16)
                vector.reciprocal_approx_fast(inv_sb[:, sl], x_sb[:, sl])
                vector._custom_dve(
                    nr_r,
                    out=r_sb[:, sl],
                    in0=x_sb[:, sl],
                    in1=inv_sb[:, sl],
                    s0=2.0,
                    s1=0.5,
                    imm2=3e38,
                )
                # pair 0 for this quarter, straight from the fresh r quarter
                # (same-engine ordering; no drain needed)
                vector._custom_dve(
                    op0,
                    out=mts[1][:, sl],
                    in0=r_sb[:, sl],
                    s0=G,
                    s1=q_ev[0],
                    imm2=q_od[0],
                ).then_inc(sem_m, 1)

            for p in range(1, NP):
                for h in range(ncol):
                    hs = slice(h * CW, (h + 1) * CW)
                    if p >= ring:
                        # mt[(p+1)%ring][:, hs] was read by ACT pair p-ring;
                        # one-deeper on its last chunk covering these columns
                        pp = p - ring
                        idx_last = ((h + 1) * counts[pp] - 1) // ncol
                        vector.wait_ge(sem_cd, cum[pp] + idx_last + 2)
                    if p % NPAIR == 0:
                        # zero-state pair: reads only r, no state tile needed
                        vector._custom_dve(
                            op0,
                            out=mts[(p + 1) % ring][:, hs],
                            in0=r_sb[:, hs],
                            s0=G,
                            s1=q_ev[p],
                            imm2=q_od[p],
                        ).then_inc(sem_m, 1)
                    else:
                        vector._custom_dve(
                            op,
                            out=mts[(p + 1) % ring][:, hs],
                            in0=mts[p % ring][:, hs],
                            in1=r_sb[:, hs],
                            s0=G,
                            s1=q_ev[p],
                            imm2=q_od[p],
                        ).then_inc(sem_m, 1)
            # sem_m fires at op completion (pre-drain); ACT therefore waits
            # one op deeper, and this trailing drain covers the last pair.
            vector.drain().then_inc(sem_m, 1)

    nc.finalize()
    return nc


def _get_nc(repeat: int = 1, ncol: int = 1, tails: int = 4,
            heads: int = 8, ring: int = 8) -> bass.Bass:
    key = (repeat, ncol, tails, heads, ring)
    if key not in _cache:
        _cache[key] = _build(repeat, ncol, tails, heads, ring)
    return _cache[key]


def _run(x: np.ndarray, repeat: int = 1):
    nc = _get_nc(repeat)
    shards = [
        np.ascontiguousarray(x[:, i * FS : (i + 1) * FS]) for i in range(NCORES)
    ]
    in_maps = [{"x": s} for s in shards]
    res = run_bass_kernel_spmd(nc, in_maps, core_ids=list(range(NCORES)))
    return [r["out"] for r in res.results]


def kernel(x: np.ndarray) -> np.ndarray:
    x = np.asarray(x, dtype=np.float32)
    outs = _run(x, repeat=1)
    code = np.concatenate(outs, axis=2)  # [B, NPAIR, F] uint8
    big = x >= 0.5  # r <= 0: spikes every step
    odd = code >= 6
    even = ((code > 2) & (code < 6)) | (odd & big[:, None, :])
    spikes = np.empty((B, T, F), dtype=np.float32)
    spikes[:, 0::2, :] = even
    spikes[:, 1::2, :] = odd
    return spikes

